# revision 15
# baseline (speedup 1.0000x reference)
"""Trainium2 Bass kernel for nn_AttentionLayer (scatter_memory).

Computation per unit u (U=512 units, sharded 64/core across 8 cores):
    query   = attention @ W[u] / temperature[u]          [B=32, 32]
    scores  = query @ keys[u]^T  (masked with -1e9)      [B, M=2048]
    weights = softmax(scores)                            -> output
    rw      = weights*rewards / sum(weights*rewards)     -> output
    outputs = rw @ mem_values[u]                         -> output

Layout: 4 units packed per 128 SBUF partitions as (u, b).  m is split
m = m1*16 + m2 (m1 = partition dim of the DMA tiles, m2 = free subblock) so
keys/mem_values load as [128, 512]-per-unit blocks with 2KB rows.  keys^T is
produced with PE fat transposes into [(u,o), m1] layout; scores / mask-add /
reward-broadcast each run as ONE full-width float32r matmul per psum bank
(fp32r requires col_grp=0xf) using block-diagonal / selector weights.
"""

import numpy as np

B, U, MS, DIN, DOUT = 32, 512, 2048, 128, 32
N_CORES = 8
U_LOC = U // N_CORES          # 64 units per core
NG = U_LOC // 4               # 16 groups of 4 units
NEG = -1.0e9

_CACHE = {}


def build(n_iters=1):
    import concourse.tile as tile
    from concourse import bacc, mybir

    f32 = mybir.dt.float32
    f32r = mybir.dt.float32r
    u8 = mybir.dt.uint8
    AF = mybir.ActivationFunctionType
    OP = mybir.AluOpType
    AX = mybir.AxisListType

    nc = bacc.Bacc(None)

    att_d = nc.declare_dram_parameter("attention", [B, DIN], f32, isOutput=False)
    w_d = nc.declare_dram_parameter("W", [U_LOC, DIN, DOUT], f32, isOutput=False)
    keys_d = nc.declare_dram_parameter("keys", [U_LOC, MS, DOUT], f32, isOutput=False)
    mv_d = nc.declare_dram_parameter("mem_values", [U_LOC, MS, DOUT], f32, isOutput=False)
    rew_d = nc.declare_dram_parameter("rewards", [U_LOC, MS], f32, isOutput=False)
    temp_d = nc.declare_dram_parameter("temperature", [U_LOC], f32, isOutput=False)
    mask_d = nc.declare_dram_parameter("mask", [B, U_LOC, MS], u8, isOutput=False)
    wts_d = nc.declare_dram_parameter("weights_out", [B, U_LOC, MS], f32, isOutput=True)
    rwo_d = nc.declare_dram_parameter("rw_out", [B, U_LOC, MS], f32, isOutput=True)
    out_d = nc.declare_dram_parameter("outputs_out", [B, U_LOC, DOUT], f32, isOutput=True)

    # constants embedded in the NEFF
    ident_d = nc.inline_tensor(np.eye(128, dtype=np.float32), name="ident128")
    negI_np = (np.float32(NEG) * np.eye(128)).astype(np.float32)
    negI_d = nc.inline_tensor(negI_np, name="negI128")
    sel_np = np.zeros((4, 128), np.float32)
    for u_ in range(4):
        sel_np[u_, 32 * u_:32 * u_ + 32] = 1.0
    sel_d = nc.inline_tensor(sel_np, name="sel4")
    zeros_d = nc.inline_tensor(np.zeros((128, 128), np.float32), name="zeros128")
    onesrow_d = nc.inline_tensor(np.ones((1, 128), np.float32), name="onesrow")

    from contextlib import ExitStack

    with tile.TileContext(nc) as tc, ExitStack() as stack:
        consts = stack.enter_context(tc.tile_pool(name="consts", bufs=1))
        ident = consts.tile([128, 128], f32, tag="ident")
        nc.sync.dma_start(out=ident[:], in_=ident_d[:])
        identr = consts.tile([128, 128], f32r, tag="identr")
        nc.sync.dma_start(out=identr[:], in_=ident_d[:].bitcast(f32r))
        negI = consts.tile([128, 128], f32r, tag="negI")
        nc.sync.dma_start(out=negI[:], in_=negI_d[:].bitcast(f32r))
        sel4 = consts.tile([4, 128], f32r, tag="sel4")
        nc.sync.dma_start(out=sel4[:], in_=sel_d[:].bitcast(f32r))
        att_sb = consts.tile([32, DIN], f32, tag="attsb")
        nc.sync.dma_start(out=att_sb[:], in_=att_d[:])
        tmp_sb = consts.tile([1, U_LOC], f32, tag="tmpsb")
        nc.sync.dma_start(out=tmp_sb[:], in_=temp_d[:].rearrange("(p u) -> p u", p=1))
        attnT = consts.tile([128, 32], f32, tag="attnT")
        invt = consts.tile([128, U_LOC], f32, tag="invt")
        onesrow = consts.tile([1, 128], f32, tag="onesrow")
        nc.sync.dma_start(out=onesrow[:], in_=onesrow_d[:])
        invt_row = consts.tile([1, U_LOC], f32, tag="invt_row")
        qts = [consts.tile([128, 128], f32r, tag=f"qt{g}", name=f"qt{g}") for g in range(NG)]
        for g in range(NG):
            # off-diagonal blocks stay zero
            nc.sync.dma_start(out=qts[g][:], in_=zeros_d[:].bitcast(f32r))

        for it in range(n_iters):
            # ---- query phase: qts[g] = blockdiag_u( W[u]^T @ attnT / temp[u] ) ----
            with tc.tile_pool(name="qps", bufs=2, space="PSUM") as qps, \
                 tc.tile_pool(name="wph", bufs=6) as wph:
                pa = qps.tile([128, 32], f32, tag="pa")
                nc.tensor.transpose(pa[:], att_sb[:], ident[:32, :32])
                nc.vector.tensor_copy(attnT[:], pa[:])
                nc.vector.reciprocal(invt_row[:], tmp_sb[:])
                # broadcast [1, U_LOC] row to all 128 partitions via K=1 matmul
                pb = qps.tile([128, U_LOC], f32, tag="pb")
                nc.tensor.matmul(pb[:], lhsT=onesrow[:], rhs=invt_row[:],
                                 start=True, stop=True, skip_group_check=True)
                nc.vector.tensor_copy(invt[:], pb[:])
                for g in range(NG):
                    pq = qps.tile([128, 128], f32, tag="pq")
                    for u in range(4):
                        uu = 4 * g + u
                        wu = wph.tile([128, 32], f32, tag="wu")
                        nc.sync.dma_start(out=wu[:], in_=w_d[uu])
                        wsc = wph.tile([128, 32], f32, tag="wsc")
                        nc.vector.tensor_scalar(wsc[:], wu[:], invt[:, uu:uu + 1], None, OP.mult)
                        nc.tensor.matmul(
                            pq[32 * u:32 * u + 32, 32 * u:32 * u + 32],
                            lhsT=wsc[:], rhs=attnT[:], start=True, stop=True,
                            tile_position=(0, 32 * u), skip_group_check=True)
                    for u in range(4):
                        nc.vector.tensor_copy(
                            qts[g][32 * u:32 * u + 32, 32 * u:32 * u + 32],
                            pq[32 * u:32 * u + 32, 32 * u:32 * u + 32])

            # ---- main pools ----
            with tc.tile_pool(name="pk", bufs=2) as pk, \
                 tc.tile_pool(name="big", bufs=2) as big, \
                 tc.tile_pool(name="ktp", bufs=6) as ktp, \
                 tc.tile_pool(name="rwtp", bufs=6) as rwtp, \
                 tc.tile_pool(name="rewp", bufs=2) as rewp, \
                 tc.tile_pool(name="scal", bufs=3) as scal, \
                 tc.tile_pool(name="psS", bufs=3, space="PSUM") as psS, \
                 tc.tile_pool(name="psT", bufs=2, space="PSUM") as psT, \
                 tc.tile_pool(name="psR", bufs=2, space="PSUM") as psR, \
                 tc.tile_pool(name="psO", bufs=1, space="PSUM") as psO:

                for g in range(NG):
                    u0 = 4 * g
                    mask_sb = big.tile([128, MS], u8, tag="mask")
                    m9 = big.tile([128, MS], f32r, tag="m9")
                    rew4 = rewp.tile([4, MS], f32r, tag="rew")
                    nc.sync.dma_start(out=rew4[:], in_=rew_d[u0:u0 + 4, :].bitcast(f32r))
                    # keys/values for 4 units in one tile: [m1, (u, m2, o)]
                    kin4 = pk.tile([128, 4 * 512], f32r, tag="kin4")
                    mv4 = pk.tile([128, 4 * 512], f32, tag="mv4")
                    for u in range(4):
                        nc.sync.dma_start(
                            out=kin4[:, 512 * u:512 * (u + 1)],
                            in_=keys_d[u0 + u].rearrange("(m1 m2) o -> m1 (m2 o)", m2=16).bitcast(f32r))
                        nc.sync.dma_start(
                            out=mv4[:, 512 * u:512 * (u + 1)],
                            in_=mv_d[u0 + u].rearrange("(m1 m2) o -> m1 (m2 o)", m2=16))
                        nc.sync.dma_start(
                            out=mask_sb[32 * u:32 * u + 32, :], in_=mask_d[:, u0 + u, :])
                    nc.scalar.activation(m9[:], mask_sb[:], AF.Copy)

                    # free-dim permute (u, m2, o) -> (m2, u, o) so each m2 block is
                    # a contiguous [m1, (u, o)] transpose input (weights APs must be
                    # single-free-dim)
                    kin4p = pk.tile([128, 4 * 512], f32r, tag="kin4p")
                    nc.scalar.activation(
                        kin4p[:].rearrange("p (m2 u o) -> p u m2 o", m2=16, u=4),
                        kin4[:].rearrange("p (u m2 o) -> p u m2 o", u=4, m2=16),
                        AF.Copy)

                    # keys^T: per m2, transpose [m1, (u,o)] -> [(u,o), m1]
                    kts = []
                    for c in range(4):
                        pt = psT.tile([128, 512], f32r, tag="pt")
                        for dm2 in range(4):
                            m2 = 4 * c + dm2
                            nc.tensor.transpose(
                                pt[:, 128 * dm2:128 * dm2 + 128],
                                kin4p[:, 128 * m2:128 * m2 + 128], identr[:])
                        kt = ktp.tile([128, 512], f32r, tag="kt")
                        nc.vector.tensor_copy(kt[:], pt[:])
                        kts.append(kt)

                    e = big.tile([128, MS], f32, tag="e")
                    f_ = big.tile([128, MS], f32, tag="f")
                    s1p = scal.tile([128, 4], f32, tag="s1p")
                    s2p = scal.tile([128, 4], f32, tag="s2p")
                    # bank c covers m = m1*16 + 4c + dm2, cols ordered (dm2, m1)
                    ev = e[:].rearrange("p (m1 c dm2) -> p c dm2 m1", c=4, dm2=4)
                    fv = f_[:].rearrange("p (m1 c dm2) -> p c dm2 m1", c=4, dm2=4)
                    m9v = m9[:].rearrange("p (m1 c dm2) -> p c dm2 m1", c=4, dm2=4)
                    rwv4 = rew4[:].rearrange("p (m1 c dm2) -> p c dm2 m1", c=4, dm2=4)

                    for c in range(4):
                        ps = psS.tile([128, 512], f32, tag="ps")
                        nc.tensor.matmul(
                            ps[:], lhsT=qts[g][:], rhs=kts[c][:],
                            start=True, stop=False, skip_group_check=True)
                        nc.tensor.matmul(
                            ps[:].rearrange("p (dm2 m1) -> p dm2 m1", dm2=4),
                            lhsT=negI[:], rhs=m9v[:, c],
                            start=False, stop=True, skip_group_check=True)
                        nc.scalar.activation(
                            ev[:, c], ps[:].rearrange("p (dm2 m1) -> p dm2 m1", dm2=4),
                            AF.Exp, accum_out=s1p[:, c:c + 1])
                        pr = psR.tile([128, 512], f32, tag="pr")
                        nc.tensor.matmul(
                            pr[:].rearrange("p (dm2 m1) -> p dm2 m1", dm2=4),
                            lhsT=sel4[:], rhs=rwv4[:, c],
                            start=True, stop=True, skip_group_check=True)
                        nc.vector.scalar_tensor_tensor(
                            out=fv[:, c], in0=ev[:, c], scalar=1.0,
                            in1=pr[:].rearrange("p (dm2 m1) -> p dm2 m1", dm2=4),
                            op0=OP.mult, op1=OP.mult,
                            accum_out=s2p[:, c:c + 1])

                    s1t = scal.tile([128, 1], f32, tag="s1t")
                    inv1 = scal.tile([128, 1], f32, tag="inv1")
                    nc.vector.tensor_reduce(s1t[:], s1p[:], axis=AX.X, op=OP.add)
                    nc.vector.reciprocal(inv1[:], s1t[:])
                    s2t = scal.tile([128, 1], f32, tag="s2t")
                    inv2 = scal.tile([128, 1], f32, tag="inv2")
                    nc.vector.tensor_reduce(s2t[:], s2p[:], axis=AX.X, op=OP.add)
                    nc.vector.reciprocal(inv2[:], s2t[:])

                    w_sb = big.tile([128, MS], f32, tag="wsb")
                    nc.vector.tensor_scalar(w_sb[:], e[:], inv1[:], None, OP.mult)
                    nc.sync.dma_start(
                        out=wts_d[:, u0:u0 + 4, :].rearrange("b u m -> u b m"), in_=w_sb[:])
                    rw_sb = big.tile([128, MS], f32, tag="rwsb")
                    nc.vector.tensor_scalar(rw_sb[:], f_[:], inv2[:], None, OP.mult)
                    nc.sync.dma_start(
                        out=rwo_d[:, u0:u0 + 4, :].rearrange("b u m -> u b m"), in_=rw_sb[:])

                    # rw^T + outputs matmuls (fp32)
                    rwv = rw_sb[:].rearrange("p (m1 m2) -> p m2 m1", m2=16)
                    rwts = []
                    for j in range(4):
                        pt2 = psT.tile([128, 512], f32, tag="pt")
                        for dm2 in range(4):
                            nc.tensor.transpose(
                                pt2[:, 128 * dm2:128 * dm2 + 128],
                                rwv[:, 4 * j + dm2], ident[:])
                        rwt = rwtp.tile([128, 512], f32, tag="rwt")
                        nc.vector.tensor_copy(rwt[:], pt2[:])
                        rwts.append(rwt)
                    # po layout: [b, (u, d)]  (col group 0, partitions 0-31)
                    po = psO.tile([32, 128], f32, tag="po")
                    for m2 in range(16):
                        j, dm2 = m2 // 4, m2 % 4
                        for u in range(4):
                            nc.tensor.matmul(
                                po[:, 32 * u:32 * u + 32],
                                lhsT=rwts[j][:, 128 * dm2 + 32 * u:128 * dm2 + 32 * u + 32],
                                rhs=mv4[:, 512 * u + 32 * m2:512 * u + 32 * m2 + 32],
                                start=(m2 == 0 and u == 0),
                                stop=(m2 == 15 and u == 3),
                                tile_position=(0, 0), skip_group_check=True)
                    out_sb = scal.tile([32, 128], f32, tag="osb")
                    nc.vector.tensor_copy(out_sb[:], po[:])
                    nc.sync.dma_start(out=out_d[:, u0:u0 + 4, :], in_=out_sb[:])

    nc.finalize()
    return nc


def _get_nc(n_iters=1):
    if n_iters not in _CACHE:
        _CACHE[n_iters] = build(n_iters)
    return _CACHE[n_iters]


def kernel(attention, W, keys, mem_values, rewards, temperature, mask, n_iters=1):
    from concourse.bass_utils import run_bass_kernel_spmd

    attention = np.ascontiguousarray(np.asarray(attention, dtype=np.float32))
    W = np.ascontiguousarray(np.asarray(W, dtype=np.float32))
    keys = np.ascontiguousarray(np.asarray(keys, dtype=np.float32))
    mem_values = np.ascontiguousarray(np.asarray(mem_values, dtype=np.float32))
    rewards = np.ascontiguousarray(np.asarray(rewards, dtype=np.float32))
    temperature = np.ascontiguousarray(np.asarray(temperature, dtype=np.float32))
    mask_u8 = np.ascontiguousarray(np.asarray(mask).astype(np.uint8))

    nc = _get_nc(n_iters)
    in_maps = []
    for c in range(N_CORES):
        sl = slice(c * U_LOC, (c + 1) * U_LOC)
        in_maps.append({
            "attention": attention,
            "W": W[sl],
            "keys": keys[sl],
            "mem_values": mem_values[sl],
            "rewards": rewards[sl],
            "temperature": temperature[sl],
            "mask": mask_u8[:, sl, :],
        })
    res = run_bass_kernel_spmd(nc, in_maps, list(range(N_CORES))).results
    weights = np.concatenate([res[c]["weights_out"] for c in range(N_CORES)], axis=1)
    rw = np.concatenate([res[c]["rw_out"] for c in range(N_CORES)], axis=1)
    outputs = np.concatenate([res[c]["outputs_out"] for c in range(N_CORES)], axis=1)
    return weights, rw, outputs
